# revision 19
# baseline (speedup 1.0000x reference)
"""Distributed multi-head attention kernel for one TRN2 chip (8 NeuronCores).

Problem: B=2, S=2048, D=1024, H=16 heads (dh=64), interleaved head split
(reshape d -> (dh, H) with heads LAST), scale = 1/sqrt(D).

Sharding: core c => batch b = c//4, head-group hg = c%4 (4 heads each).
No collectives: every core computes its own [s, 256] output slice.

Host-side marshalling:
  - weight columns permuted so each head's 64 columns are contiguous
  - x[b] pre-transposed to xT [D, S] (PE contracts over partitions, so x
    must be d-major; transposing on host is free)
  - bf16 casts for all matmul operands (fp32 PE matmul is multi-pass slow)

Device-side (per core, SPMD):
  - QT[dq, s] = Wq.T @ x.T (+bias), KT likewise, V[s, dv] with an extra
    ones column per head (V_aug) so PV also produces softmax row sums
  - per head: ST[j, i] = KT_h.T @ QT_h (K=64); E = exp(ST/32) on ScalarE
    straight out of PSUM (scores are tiny, |s| < ~0.3, so softmax without
    max subtraction is numerically safe)
  - OT_aug[65, i] += V_aug[j,:].T @ E[j, i] accumulated over j tiles;
    row 64 = sum_j E (softmax denominator)
  - normalize: DVE reciprocal of row 64, GPSIMD partition_broadcast,
    DVE multiply; DMA out OT [64(c), 2048(s)] per head (host transposes)
"""

import sys
import os

for _p in ("/opt/trn_rl_repo",):
    if os.path.isdir(_p) and _p not in sys.path:
        sys.path.insert(0, _p)

import numpy as np
import ml_dtypes
from contextlib import ExitStack

import concourse.bass as bass
import concourse.mybir as mybir
import concourse.tile as tile
from concourse import bacc
from concourse.bass_utils import run_bass_kernel_spmd

BF16 = mybir.dt.bfloat16
F32 = mybir.dt.float32
NPBF16 = ml_dtypes.bfloat16

B, S, D, H = 2, 2048, 1024, 16
NCORES = 8
HGROUPS = 4              # tensor-parallel ways over heads
NH_LOC = H // HGROUPS    # 4 heads per core
DH = D // H              # 64
DQ = NH_LOC * DH         # 256 projection cols per core
KT = D // 128            # 8 contraction tiles
SCALE = 1.0 / 32.0       # 1/sqrt(D)

# column permutation: permuted col h*64+c  <-  original col c*16+h
PERM = np.array([c * H + h for h in range(H) for c in range(DH)], dtype=np.int64)


def build_bass():
    nc = bacc.Bacc("TRN2", target_bir_lowering=False)
    xT_d = nc.dram_tensor("xT", [D, S], BF16, kind="ExternalInput")
    wq_d = nc.dram_tensor("wq", [D, DQ], BF16, kind="ExternalInput")
    wk_d = nc.dram_tensor("wk", [D, DQ], BF16, kind="ExternalInput")
    wv_d = nc.dram_tensor("wv", [D, DQ], BF16, kind="ExternalInput")
    bqT_d = nc.dram_tensor("bqT", [DQ, 1], F32, kind="ExternalInput")
    out_d = nc.dram_tensor("out", [DQ, S], F32, kind="ExternalOutput")

    with ExitStack() as ctx:
        tc = ctx.enter_context(tile.TileContext(nc))
        consts = ctx.enter_context(tc.tile_pool(name="consts", bufs=1))
        xpool = ctx.enter_context(tc.tile_pool(name="xpool", bufs=KT))
        epool = ctx.enter_context(tc.tile_pool(name="epool", bufs=10))
        npool = ctx.enter_context(tc.tile_pool(name="npool", bufs=2))
        opool = ctx.enter_context(tc.tile_pool(name="opool", bufs=2))
        # PSUM is exactly 16KB/partition = 8 banks. Layout (per partition):
        #   psS  2x4KB — double-buffered score tiles [128,1024]
        #   psPJ 2x2KB — projection accumulator / dummy warm-keeper slot
        #   pov  4KB   — PV accumulator [65,1024] for the current i-half
        psS = ctx.enter_context(tc.tile_pool(name="psS", bufs=2, space="PSUM"))
        psPJ = ctx.enter_context(tc.tile_pool(name="psPJ", bufs=2, space="PSUM"))
        pov = ctx.enter_context(tc.tile_pool(name="pov", bufs=1, space="PSUM"))

        # ---- input DMAs (ordered by first use) ----
        xT_sb = [xpool.tile([128, S], BF16, tag="xT", name=f"xT{_i}") for _i in range(KT)]
        wq_sb = consts.tile([128, KT, DQ], BF16)
        wk_sb = consts.tile([128, KT, DQ], BF16)
        wv_sb = consts.tile([128, KT, DQ], BF16)
        bq_sb = consts.tile([128, 2, 1], F32)
        nc.sync.dma_start(out=wq_sb[:], in_=wq_d.ap().rearrange("(t p) n -> p t n", p=128))
        for kt in range(KT):
            nc.sync.dma_start(out=xT_sb[kt][:, 0:512],
                              in_=xT_d[kt * 128:(kt + 1) * 128, 0:512])
        nc.sync.dma_start(out=wk_sb[:], in_=wk_d.ap().rearrange("(t p) n -> p t n", p=128))
        for kt in range(KT):
            nc.sync.dma_start(out=xT_sb[kt][:, 512:1024],
                              in_=xT_d[kt * 128:(kt + 1) * 128, 512:1024])
        nc.sync.dma_start(out=bq_sb[:], in_=bqT_d.ap().rearrange("(t p) o -> p t o", p=128))
        nc.sync.dma_start(out=wv_sb[:], in_=wv_d.ap().rearrange("(t p) n -> p t n", p=128))
        for q in range(2, 4):
            for kt in range(KT):
                nc.sync.dma_start(out=xT_sb[kt][:, q * 512:(q + 1) * 512],
                                  in_=xT_d[kt * 128:(kt + 1) * 128, q * 512:(q + 1) * 512])

        qt_sb = consts.tile([128, 2, S], BF16)
        kt_sb = consts.tile([128, 2, S], BF16)
        v_sb = consts.tile([128, 16, NH_LOC * (DH + 1)], BF16)

        # warm the PE clock gate (HAM) with dummy matmuls while DMAs run;
        # ~3.4us of sustained activity moves the PE from 1.2 to 2.4 GHz
        warm_in = consts.tile([128, 512], BF16)
        nc.gpsimd.memset(warm_in[:], 0.0)
        nc.vector.memset(v_sb[:], 1.0)
        warm_ps = pov.tile([65, 1024], F32, tag="ov", name="warmps")
        for w in range(8):
            nc.tensor.matmul(warm_ps[:, 0:512], lhsT=warm_in[:, 0:DH + 1], rhs=warm_in[:],
                             start=(w == 0), stop=(w == 7))

        # ---- projection chunk emitters (each: 8 accumulating MMs + evict) ----
        def proj_q(m, ic):
            ps = psPJ.tile([128, 512], F32, tag="pj", name="psq")
            for kt in range(KT):
                nc.tensor.matmul(
                    ps[:], lhsT=wq_sb[:, kt, m * 128:(m + 1) * 128],
                    rhs=xT_sb[kt][:, ic * 512:(ic + 1) * 512],
                    start=(kt == 0), stop=(kt == KT - 1))
            nc.vector.tensor_scalar_add(
                qt_sb[:, m, ic * 512:(ic + 1) * 512], ps[:], bq_sb[:, m, :])

        def proj_k(m, ic):
            ps = psPJ.tile([128, 512], F32, tag="pj", name="psk")
            for kt in range(KT):
                nc.tensor.matmul(
                    ps[:], lhsT=wk_sb[:, kt, m * 128:(m + 1) * 128],
                    rhs=xT_sb[kt][:, ic * 512:(ic + 1) * 512],
                    start=(kt == 0), stop=(kt == KT - 1))
            nc.vector.tensor_copy(out=kt_sb[:, m, ic * 512:(ic + 1) * 512], in_=ps[:])

        def proj_v(st):
            ps = psPJ.tile([128, 512], F32, tag="pj", name="psv")
            for kt in range(KT):
                nc.tensor.matmul(
                    ps[:, 0:DQ], lhsT=xT_sb[kt][:, st * 128:(st + 1) * 128],
                    rhs=wv_sb[:, kt, :], start=(kt == 0), stop=(kt == KT - 1))
            nc.vector.tensor_copy(
                out=v_sb[:, st, :].rearrange("p (h e) -> p h e", e=DH + 1)[:, :, 0:DH],
                in_=ps[:, 0:DQ].rearrange("p (h c) -> p h c", c=DH))

        def dummy_mm():
            ps = psPJ.tile([128, 512], F32, tag="pj", name="dum")
            nc.tensor.matmul(ps[:], lhsT=warm_in[:, 0:128], rhs=warm_in[:],
                             start=True, stop=True)

        EXP = mybir.ActivationFunctionType.Exp

        # prologue: the three projection chunks the first exp needs
        proj_q(0, 0)
        proj_k(0, 0)
        proj_q(0, 1)
        for st in range(4):
            proj_v(st)

        # per head, two i-half passes; per (pass, jc): one [128,1024] score
        # tile -> one exp -> two PV accumulations into the [65,1024] o_ph
        for h in range(NH_LOC):
            m = h // 2
            off = (h % 2) * DH
            off_sl = slice(off, off + DH)
            for ih in range(2):
                ibase = ih * 1024
                o_ph = pov.tile([DH + 1, 1024], F32, tag="ov", name="oph")
                for jc in range(16):
                    # scores first at high priority: the scheduler must always
                    # prefer feeding ScalarE over projection bursts
                    with tc.high_priority():
                        ps = psS.tile([128, 1024], F32, tag="sS", name="ss")
                        for i2 in range(2):
                            nc.tensor.matmul(
                                ps[:, i2 * 512:(i2 + 1) * 512],
                                lhsT=kt_sb[off_sl, m, jc * 128:(jc + 1) * 128],
                                rhs=qt_sb[off_sl, m, ibase + i2 * 512:ibase + (i2 + 1) * 512],
                                start=True, stop=True)
                        e_sb = epool.tile([128, 1024], BF16, tag="e", name="esb")
                        nc.scalar.activation(e_sb[:], ps[:], EXP, scale=SCALE)

                    # interleaved projection work on the dedicated PJ slot,
                    # issued while ScalarE runs the exp
                    if h == 0 and ih == 0:
                        if jc == 1:
                            proj_k(0, 1)
                        elif jc == 5:
                            proj_k(0, 2)
                        elif jc == 9:
                            proj_k(0, 3)
                        elif jc == 13:
                            proj_q(0, 2)
                        elif jc == 14:
                            proj_q(0, 3)
                        if jc < 12:
                            proj_v(jc + 4)    # V(0..3) done in prologue
                    elif h == 1 and jc % 4 == 0:
                        i2 = (ih * 16 + jc) // 4
                        (proj_q if i2 % 2 == 0 else proj_k)(1, i2 // 2)

                    for i2 in range(2):
                        nc.tensor.matmul(
                            o_ph[:, i2 * 512:(i2 + 1) * 512],
                            lhsT=v_sb[:, jc, h * (DH + 1):(h + 1) * (DH + 1)],
                            rhs=e_sb[:, i2 * 512:(i2 + 1) * 512],
                            start=(jc == 0), stop=(jc == 15))

                # normalize this i-half; plain copies release the accumulator,
                # the rest runs from SBUF off the PE/ACT critical path
                sl = slice(ibase, ibase + 1024)
                o_sb = opool.tile([DH, 1024], F32, tag="osb")
                rl_sb = npool.tile([1, 1024], F32, tag="rl")
                rb_sb = npool.tile([DH, 1024], F32, tag="rb")
                rl2_sb = npool.tile([1, 1024], F32, tag="rl2")
                ost = opool.tile([DH, 1024], F32, tag="ost")
                nc.vector.tensor_copy(out=rl_sb[:], in_=o_ph[DH:DH + 1, :])
                nc.vector.tensor_copy(out=o_sb[:], in_=o_ph[0:DH, :])
                nc.vector.reciprocal_approx_fast(out=rl2_sb[:], in_=rl_sb[:])
                nc.gpsimd.partition_broadcast(rb_sb[:], rl2_sb[:])
                nc.vector.tensor_mul(ost[:], o_sb[:], rb_sb[:])
                nc.sync.dma_start(out=out_d[h * DH:(h + 1) * DH, sl], in_=ost[:])

    nc.finalize()
    return nc


_NC_CACHE = None


def _get_nc():
    global _NC_CACHE
    if _NC_CACHE is None:
        _NC_CACHE = build_bass()
    return _NC_CACHE


def kernel(x, Wq, Bq, Wk, Wv, n_heads=16, **_ignored):
    x = np.asarray(x, dtype=np.float32)
    Wq = np.asarray(Wq, dtype=np.float32)
    Bq = np.asarray(Bq, dtype=np.float32).reshape(-1)
    Wk = np.asarray(Wk, dtype=np.float32)
    Wv = np.asarray(Wv, dtype=np.float32)

    wq_p = Wq[:, PERM]
    wk_p = Wk[:, PERM]
    wv_p = Wv[:, PERM]
    bq_p = Bq[PERM]

    xT = [np.ascontiguousarray(x[b].T).astype(NPBF16) for b in range(B)]
    in_maps = []
    for core in range(NCORES):
        b, hg = core // HGROUPS, core % HGROUPS
        sl = slice(hg * DQ, (hg + 1) * DQ)
        in_maps.append({
            "xT": xT[b],
            "wq": np.ascontiguousarray(wq_p[:, sl]).astype(NPBF16),
            "wk": np.ascontiguousarray(wk_p[:, sl]).astype(NPBF16),
            "wv": np.ascontiguousarray(wv_p[:, sl]).astype(NPBF16),
            "bqT": np.ascontiguousarray(bq_p[sl]).reshape(DQ, 1).astype(np.float32),
        })

    nc = _get_nc()
    res = run_bass_kernel_spmd(nc, in_maps, core_ids=list(range(NCORES)))

    out = np.empty((B, S, D), dtype=np.float32)
    for b in range(B):
        big = np.concatenate(
            [res.results[b * HGROUPS + hg]["out"] for hg in range(HGROUPS)], axis=0)
        out[b][:, PERM] = big.T
    return out
